# revision 17
# baseline (speedup 1.0000x reference)
"""Trainium2 Bass kernel for nn_CrossAttention (B=2, S=2048, E=1024, H=16, ctx=768).

Sharding: 4-way tensor-parallel over heads x 2-way data-parallel over batch.
Core c handles batch c//4 and heads 4*(c%4) .. 4*(c%4)+3.

v3: fully-interleaved single pipeline. The softmax EXP on ScalarE
(128 x ~1016ns = 130us) is the per-iteration rate limiter, and total PE
work (~131us) nearly equals total ACT work, so the kernel emits one
p-major chunk stream ((p,sc) for p in 0,1 for sc in 0..3) where every
exp-gated iteration also carries background PE work (remaining
projections, out-projection) fed by a just-in-time rate-smoothed driver
(deadline in linear iteration time, ~1 quantum/iter lookahead).  Input
DMAs are column-grouped so the first attention chunk's operands land on
many HW queues in parallel (a single [128,2048]f16 tile DMA is ~23us on
one ~22GB/s queue); the out-projection result is written back as fp16
in two half-tiles per block for the same reason.

Per-core dataflow (fp16 operands, fp32 PSUM):
  qT/kT = W-stationary projections producing [dh, S] layouts
  v     = ctxT-tile-stationary, per g-half, bias folded into the DVE add
  scT   = kT x qT pairs on PE row groups 0/64 (concurrent)
  exp   = ScalarE, scale=0.125, PSUM -> SBUF fp16
  av/Z  = [v_h | ones] stationary -> av.T rows 0:64, Z rows 64:128
  out   = avT-stationary x Wo, interleaved as filler quanta

Host side: pre-transpose x/context, slice weights per head group, fp16;
bv pre-replicated to [128, DSL]; sum the 4 per-batch partials + bo.
"""
import numpy as np

import concourse.bass as bass
import concourse.mybir as mybir
import concourse.tile as tile
from concourse import bacc, bass_utils

F16 = mybir.dt.float16
F32 = mybir.dt.float32
AF = mybir.ActivationFunctionType
OP = mybir.AluOpType

B, S, E, C, H, DH = 2, 2048, 1024, 768, 16, 64
N_CORES = 8
GROUPS = 4            # head groups (tensor parallel)
HPG = H // GROUPS     # heads per group = 4
DSL = HPG * DH        # feature slice per core = 256
KT_E = E // 128       # 8 k-tiles for x projections
KT_C = C // 128       # 6 k-tiles for context projections
SCK = S // 512        # 4 s-chunks
TT = S // 128         # 16 t-tiles

_NC_CACHE = {}


def _build_nc():
    nc = bacc.Bacc("TRN2", target_bir_lowering=False, debug=False,
                   num_devices=N_CORES)

    xT = nc.dram_tensor("xT", [E, S], F16, kind="ExternalInput").ap()
    ctxT = nc.dram_tensor("ctxT", [C, S], F16, kind="ExternalInput").ap()
    wq = nc.dram_tensor("wq", [E, DSL], F16, kind="ExternalInput").ap()
    wk = nc.dram_tensor("wk", [C, DSL], F16, kind="ExternalInput").ap()
    wv = nc.dram_tensor("wv", [C, DSL], F16, kind="ExternalInput").ap()
    wo = nc.dram_tensor("wo", [DSL, E], F16, kind="ExternalInput").ap()
    bq = nc.dram_tensor("bq", [128, 2], F32, kind="ExternalInput").ap()
    bk = nc.dram_tensor("bk", [128, 2], F32, kind="ExternalInput").ap()
    bv = nc.dram_tensor("bv", [128, DSL], F32, kind="ExternalInput").ap()
    out = nc.dram_tensor("out", [S, E], F16, kind="ExternalOutput").ap()

    xT_r = xT.rearrange("(o p) s -> p o s", p=128)
    ctxT_r = ctxT.rearrange("(o p) s -> p o s", p=128)
    wq_r = wq.rearrange("(o p) m -> p o m", p=128)
    wk_r = wk.rearrange("(o p) m -> p o m", p=128)
    wv_r = wv.rearrange("(o p) m -> p o m", p=128)
    wo_r = wo.rearrange("(l p) n -> p l n", p=128)

    with tile.TileContext(nc) as tc:
        with (
            tc.tile_pool(name="const", bufs=1) as cpool,
            tc.tile_pool(name="qkv", bufs=1) as qpool,
            tc.tile_pool(name="ex", bufs=6) as expool,
            tc.tile_pool(name="os", bufs=3) as ospool,
        ):
            wk_sb = cpool.tile([128, KT_C, DSL], F16)
            wv_sb = cpool.tile([128, KT_C, DSL], F16)
            wq_sb = cpool.tile([128, KT_E, DSL], F16)
            wo_sb = cpool.tile([128, 2, E], F16)
            bq_sb = cpool.tile([128, 2], F32)
            bk_sb = cpool.tile([128, 2], F32)
            bv_sb = cpool.tile([128, DSL], F32)
            warm_sb = cpool.tile([1, 8], F32)
            ctxT_sb = cpool.tile([128, KT_C, S], F16)
            xT_sb = cpool.tile([128, KT_E, S], F16)

            qT_sb = qpool.tile([128, 2, S], F16)
            kT_sb = qpool.tile([128, 2, S], F16)
            # per (t, head): 128 cols = [v_h (64) | ones (64)] so one matmul
            # yields av rows 0:64 and the replicated softmax denominator
            # rows 64:128 in a single PSUM bank
            v2_sb = qpool.tile([128, TT, HPG, 128], F16)
            avT_sb = qpool.tile([128, 2, S], F16)
            # ones halves of v2; on GpSimd so it gates nothing else
            nc.gpsimd.memset(v2_sb[:], 1.0)
            nc.vector.memset(warm_sb[:], 0.0)
            # pull the exp table load off the critical path
            nc.scalar.activation(warm_sb[:], warm_sb[:], AF.Exp)

            # ---- input DMAs: weights first (small), then x/ctx in column
            # groups so the first group's 14 tiles ride 14 HW queues ----
            for k in range(KT_C):
                nc.sync.dma_start(wk_sb[:, k, :], wk_r[:, k, :])
            nc.sync.dma_start(bk_sb[:], bk[:])
            for k in range(KT_C):
                nc.sync.dma_start(wv_sb[:, k, :], wv_r[:, k, :])
            nc.sync.dma_start(bv_sb[:], bv[:])
            for k in range(KT_E):
                nc.sync.dma_start(wq_sb[:, k, :], wq_r[:, k, :])
            nc.sync.dma_start(bq_sb[:], bq[:])
            for cg in range(4):
                csl = slice(cg * 512, (cg + 1) * 512)
                for k in range(KT_C):
                    nc.sync.dma_start(ctxT_sb[:, k, csl], ctxT_r[:, k, csl])
                for k in range(KT_E):
                    nc.sync.dma_start(xT_sb[:, k, csl], xT_r[:, k, csl])
                if cg == 1:
                    for wg in range(2):
                        wsl = slice(wg * 512, (wg + 1) * 512)
                        for l in range(2):
                            nc.sync.dma_start(wo_sb[:, l, wsl],
                                              wo_r[:, l, wsl])

            # HAM warm-up: dependency-free matmul chain keeps the PE busy
            # through the preamble + input-DMA stream so the first real
            # matmuls run at 2.4GHz instead of 1.2
            wrm_in = cpool.tile([128, 256], F16)
            nc.vector.memset(wrm_in[:], 0.5)

            with (
                tc.tile_pool(name="psc", bufs=2, space="PSUM") as psc,
                tc.tile_pool(name="pacc", bufs=2, space="PSUM") as pacc,
                tc.tile_pool(name="pbg", bufs=2, space="PSUM") as pbg,
            ):
                wrm_ps = pbg.tile([128, 256], F32, tag="bg", name="wrmps")
                for i in range(30):
                    nc.tensor.matmul(wrm_ps[:], wrm_in[:, 0:128],
                                     wrm_in[:], start=(i == 0),
                                     stop=(i == 29))
                # ---- background-work generators (1 yield = 1-2 matmuls) ----
                def proj_gen(dst, w_sb, b_sb, src, nk, l, sc, nm):
                    # produces dst[:, l, sc*512:(sc+1)*512]
                    pss = pbg.tile([128, 512], F32, tag="bg",
                                   name=f"pj{nm}_{l}_{sc}")
                    for k in range(nk):
                        nc.tensor.matmul(
                            pss[:],
                            w_sb[:, k, l * 128:(l + 1) * 128],
                            src[:, k, sc * 512:(sc + 1) * 512],
                            start=(k == 0), stop=(k == nk - 1),
                        )
                        yield
                    nc.vector.tensor_tensor(
                        dst[:, l, sc * 512:(sc + 1) * 512],
                        pss[:],
                        b_sb[:, l:l + 1].to_broadcast([128, 512]),
                        OP.add,
                    )
                    yield

                def v_gen(t, half):
                    # v2[:, t, 2h:2h+2, 0:64] = ctx_t @ wv[:, half] + bv
                    ps = pbg.tile([128, 128], F32, tag="bg",
                                  name=f"vps{t}_{half}")
                    for k in range(KT_C):
                        nc.tensor.matmul(
                            ps[:],
                            ctxT_sb[:, k, t * 128:(t + 1) * 128],
                            wv_sb[:, k, half * 128:(half + 1) * 128],
                            start=(k == 0), stop=(k == KT_C - 1),
                        )
                        if k % 2 == 1:
                            yield
                    nc.vector.tensor_tensor(
                        v2_sb[:, t, 2 * half:2 * half + 2, 0:DH],
                        ps[:].rearrange("p (g d) -> p g d", d=DH),
                        bv_sb[:, half * 128:(half + 1) * 128].rearrange(
                            "p (g d) -> p g d", d=DH),
                        OP.add,
                    )
                    yield

                def outproj_gen(sc):
                    for st in range(4):
                        row = (sc * 4 + st) * 128
                        for n in range(2):
                            ps = pbg.tile([128, 512], F32, tag="bg",
                                          name=f"po{sc}_{st}_{n}")
                            for l in range(2):
                                nc.tensor.matmul(
                                    ps[:],
                                    avT_sb[:, l, row:row + 128],
                                    wo_sb[:, l, n * 512:(n + 1) * 512],
                                    start=(l == 0), stop=(l == 1),
                                )
                            yield
                            os_sb = ospool.tile([128, 512], F16, tag="os")
                            nc.vector.tensor_copy(os_sb[:], ps[:])
                            for half in range(2):
                                hsl = slice(n * 512 + half * 256,
                                            n * 512 + (half + 1) * 256)
                                nc.sync.dma_start(
                                    out[row:row + 128, hsl],
                                    os_sb[:, half * 256:(half + 1) * 256],
                                )
                            yield

                # work items: [deadline_T, generator, quanta_left]
                # deadline_T in linear iteration time T = ci*16 + t; the gen
                # must be fully emitted before iteration T begins.
                work = []
                for sc in range(SCK):
                    work.append([4 * sc,
                                 proj_gen(kT_sb, wk_sb, bk_sb, ctxT_sb,
                                          KT_C, 0, sc, "k"), KT_C + 1])
                for t in range(TT):
                    work.append([t, v_gen(t, 0), 4])
                for sc in range(SCK):
                    work.append([max(0, 16 * sc - 8),
                                 proj_gen(qT_sb, wq_sb, bq_sb, xT_sb,
                                          KT_E, 0, sc, "q"), KT_E + 1])
                # l1 work pulled into chunks 1-3 so chunk 4 isn't an
                # avalanche of forced drains
                for sc in range(SCK):
                    work.append([18 + 10 * sc,
                                 proj_gen(kT_sb, wk_sb, bk_sb, ctxT_sb,
                                          KT_C, 1, sc, "k"), KT_C + 1])
                for t in range(TT):
                    work.append([20 + (5 * t + 1) // 2, v_gen(t, 1), 4])
                for sc in range(SCK):
                    work.append([52 + 16 * sc,
                                 proj_gen(qT_sb, wq_sb, bq_sb, xT_sb,
                                          KT_E, 1, sc, "q"), KT_E + 1])

                def advance(w, n):
                    for _ in range(n):
                        try:
                            next(w[1])
                            w[2] -= 1
                        except StopIteration:
                            w[2] = 0
                            break
                    if w[2] <= 0:
                        try:
                            for _ in w[1]:
                                pass
                        except StopIteration:
                            pass
                        return False
                    return True

                def drive(T):
                    # just-in-time: keep every gen's remaining quanta within
                    # its slack (1 quantum/iter burn rate), earliest first
                    work.sort(key=lambda w: w[0])
                    for w in list(work):
                        slack = w[0] - T - 1
                        if slack <= 0:
                            advance(w, 10 ** 6)
                            work.remove(w)
                        elif w[2] > slack:
                            if not advance(w, w[2] - slack):
                                work.remove(w)

                def drive_one(T):
                    if work:
                        if not advance(work[0], 1):
                            work.remove(work[0])

                # ---- main p-major chunk loop -------------------------------
                for ci, (p, sc) in enumerate(
                        [(p, sc) for p in (0, 1) for sc in range(SCK)]):
                    ssl = slice(sc * 512, (sc + 1) * 512)
                    avz = {h: pacc.tile([128, 512], F32, tag="acc",
                                        name=f"avz{ci}_{h}")
                           for h in range(2)}
                    exq = []

                    def emit_av(tt_):
                        ex_ = exq.pop(0)
                        for h in range(2):
                            nc.tensor.matmul(
                                avz[h][:, :],
                                v2_sb[:, tt_, p * 2 + h, :],
                                ex_[:, h * 512:(h + 1) * 512],
                                start=(tt_ == 0), stop=(tt_ == TT - 1),
                            )

                    for t in range(TT):
                        drive(ci * 16 + t)
                        # both heads in one 2-bank tile: h0 cols 0:512,
                        # h1 cols 512:1024 — emitted back-to-back on PE
                        # row groups 0/64 so they run concurrently
                        scp = psc.tile([128, 1024], F32, tag="sc",
                                       name=f"sc{ci}_{t}")
                        for h in range(2):
                            hb = h * DH
                            nc.tensor.matmul(
                                scp[:, h * 512:(h + 1) * 512],
                                kT_sb[hb:hb + DH, p, t * 128:(t + 1) * 128],
                                qT_sb[hb:hb + DH, p, ssl],
                                start=True, stop=True,
                            )
                        ex = expool.tile([128, 1024], F16, tag="ex",
                                         name=f"ex{ci}_{t}")
                        nc.scalar.activation(ex[:], scp[:], AF.Exp,
                                             scale=0.125)
                        exq.append(ex)
                        # av lagged 2 iterations: by emission time its exp
                        # is long done, so the PE never stalls on the fresh
                        # exp and the av LDWEIGHTS backgrounds fully
                        if t >= 2:
                            emit_av(t - 2)
                        drive_one(ci * 16 + t)
                    emit_av(TT - 2)
                    emit_av(TT - 1)
                    for h in range(2):
                        hb = h * DH
                        # custom DVE op: SBUF-only, partition base 0
                        rz = ospool.tile([128, 1024], F32, tag="rz",
                                         name=f"rz{ci}_{h}")
                        nc.vector.tensor_copy(
                            rz[0:DH, 0:512], avz[h][DH:128, :])
                        nc.vector.reciprocal_approx_fast(
                            rz[0:DH, 512:1024], rz[0:DH, 0:512])
                        nc.vector.tensor_tensor(
                            avT_sb[hb:hb + DH, p, ssl],
                            avz[h][0:DH, :],
                            rz[0:DH, 512:1024],
                            OP.mult,
                        )
                    if p == 1:
                        # avT rows for this sc now complete in both l slices
                        dl = (5 + sc) * 16 + 15 if sc < 3 else 10 ** 6
                        work.append([dl, outproj_gen(sc), 16])
                # drain remaining outproj work (tail)
                for w in list(work):
                    advance(w, 10 ** 6)

    nc.compile()
    return nc


def get_nc():
    if "nc" not in _NC_CACHE:
        _NC_CACHE["nc"] = _build_nc()
    return _NC_CACHE["nc"]


def make_in_maps(x, context, Wq, bq, Wk, bk, Wv, bv, Wo, bo):
    x = np.asarray(x, dtype=np.float32)
    context = np.asarray(context, dtype=np.float32)
    Wq = np.asarray(Wq, dtype=np.float32)
    Wk = np.asarray(Wk, dtype=np.float32)
    Wv = np.asarray(Wv, dtype=np.float32)
    Wo = np.asarray(Wo, dtype=np.float32)
    bq = np.asarray(bq, dtype=np.float32)
    bk = np.asarray(bk, dtype=np.float32)
    bv = np.asarray(bv, dtype=np.float32)

    xT = [np.ascontiguousarray(x[b].T).astype(np.float16) for b in range(B)]
    ctxT = [np.ascontiguousarray(context[b].T).astype(np.float16)
            for b in range(B)]
    in_maps = []
    for c in range(N_CORES):
        b, g = c // GROUPS, c % GROUPS
        sl = slice(g * DSL, (g + 1) * DSL)
        in_maps.append({
            "xT": xT[b],
            "ctxT": ctxT[b],
            "wq": Wq[:, sl].astype(np.float16),
            "wk": Wk[:, sl].astype(np.float16),
            "wv": Wv[:, sl].astype(np.float16),
            "wo": Wo[sl, :].astype(np.float16),
            "bq": np.ascontiguousarray(bq[sl].reshape(2, 128).T),
            "bk": np.ascontiguousarray(bk[sl].reshape(2, 128).T),
            "bv": np.tile(bv[sl].reshape(1, DSL).astype(np.float32),
                          (128, 1)),
        })
    return in_maps


def run_sharded(inputs, trace=False):
    nc = get_nc()
    in_maps = make_in_maps(**inputs)
    res = bass_utils.run_bass_kernel_spmd(
        nc, in_maps, core_ids=list(range(N_CORES)), trace=trace,
    )
    bo = np.asarray(inputs["bo"], dtype=np.float32)
    full = np.empty((B, S, E), dtype=np.float32)
    for b in range(B):
        acc = res.results[b * GROUPS]["out"].astype(np.float32)
        for g in range(1, GROUPS):
            acc = acc + res.results[b * GROUPS + g]["out"].astype(np.float32)
        full[b] = acc + bo[None, :]
    return full, res.exec_time_ns


def kernel(**inputs) -> np.ndarray:
    return run_sharded(inputs)[0]


# revision 21
# speedup vs baseline: 1.0623x; 1.0623x over previous
"""Trainium2 Bass kernel for nn_CrossAttention (B=2, S=2048, E=1024, H=16, ctx=768).

Sharding: 4-way tensor-parallel over heads x 2-way data-parallel over batch.
Core c handles batch c//4 and heads 4*(c%4) .. 4*(c%4)+3.

v3: fully-interleaved single pipeline. The softmax EXP on ScalarE
(128 x ~1016ns = 130us) is the per-iteration rate limiter, and total PE
work (~131us) nearly equals total ACT work, so the kernel emits one
p-major chunk stream ((p,sc) for p in 0,1 for sc in 0..3) where every
exp-gated iteration also carries background PE work (remaining
projections, out-projection) fed by a just-in-time rate-smoothed driver
(deadline in linear iteration time, ~1 quantum/iter lookahead).  Input
DMAs are column-grouped so the first attention chunk's operands land on
many HW queues in parallel (a single [128,2048]f16 tile DMA is ~23us on
one ~22GB/s queue); the out-projection result is written back as fp16
in two half-tiles per block for the same reason.

Per-core dataflow (fp16 operands, fp32 PSUM):
  qT/kT = W-stationary projections producing [dh, S] layouts
  v     = ctxT-tile-stationary, per g-half, bias folded into the DVE add
  scT   = kT x qT pairs on PE row groups 0/64 (concurrent)
  exp   = ScalarE, scale=0.125, PSUM -> SBUF fp16
  av/Z  = [v_h | ones] stationary -> av.T rows 0:64, Z rows 64:128
  out   = avT-stationary x Wo, interleaved as filler quanta

Host side: pre-transpose x/context, slice weights per head group, fp16;
bv pre-replicated to [128, DSL]; sum the 4 per-batch partials + bo.
"""
import numpy as np

import concourse.bass as bass
import concourse.mybir as mybir
import concourse.tile as tile
from concourse import bacc, bass_utils

F16 = mybir.dt.float16
F32 = mybir.dt.float32
AF = mybir.ActivationFunctionType
OP = mybir.AluOpType

B, S, E, C, H, DH = 2, 2048, 1024, 768, 16, 64
N_CORES = 8
GROUPS = 4            # head groups (tensor parallel)
HPG = H // GROUPS     # heads per group = 4
DSL = HPG * DH        # feature slice per core = 256
KT_E = E // 128       # 8 k-tiles for x projections
KT_C = C // 128       # 6 k-tiles for context projections
SCK = S // 512        # 4 s-chunks
TT = S // 128         # 16 t-tiles

_NC_CACHE = {}


def _build_nc():
    nc = bacc.Bacc("TRN2", target_bir_lowering=False, debug=False,
                   num_devices=N_CORES)

    xT = nc.dram_tensor("xT", [E, S], F16, kind="ExternalInput").ap()
    ctxT = nc.dram_tensor("ctxT", [C, S], F16, kind="ExternalInput").ap()
    wq = nc.dram_tensor("wq", [E, DSL], F16, kind="ExternalInput").ap()
    wk = nc.dram_tensor("wk", [C, DSL], F16, kind="ExternalInput").ap()
    wv = nc.dram_tensor("wv", [C, DSL], F16, kind="ExternalInput").ap()
    wo = nc.dram_tensor("wo", [DSL, E], F16, kind="ExternalInput").ap()
    bq = nc.dram_tensor("bq", [128, 2], F32, kind="ExternalInput").ap()
    bk = nc.dram_tensor("bk", [128, 2], F32, kind="ExternalInput").ap()
    bv = nc.dram_tensor("bv", [128, DSL], F32, kind="ExternalInput").ap()
    out = nc.dram_tensor("out", [S, E], F16, kind="ExternalOutput").ap()

    xT_r = xT.rearrange("(o p) s -> p o s", p=128)
    ctxT_r = ctxT.rearrange("(o p) s -> p o s", p=128)
    wq_r = wq.rearrange("(o p) m -> p o m", p=128)
    wk_r = wk.rearrange("(o p) m -> p o m", p=128)
    wv_r = wv.rearrange("(o p) m -> p o m", p=128)
    wo_r = wo.rearrange("(l p) n -> p l n", p=128)

    with tile.TileContext(nc) as tc:
        with (
            tc.tile_pool(name="const", bufs=1) as cpool,
            tc.tile_pool(name="qkv", bufs=1) as qpool,
            tc.tile_pool(name="ex", bufs=8) as expool,
            tc.tile_pool(name="os", bufs=3) as ospool,
        ):
            wk_sb = cpool.tile([128, KT_C, DSL], F16)
            wv_sb = cpool.tile([128, KT_C, DSL], F16)
            wq_sb = cpool.tile([128, KT_E, DSL], F16)
            wo_sb = cpool.tile([128, 2, E], F16)
            bq_sb = cpool.tile([128, 2], F32)
            bk_sb = cpool.tile([128, 2], F32)
            bv_sb = cpool.tile([128, DSL], F32)
            warm_sb = cpool.tile([1, 8], F32)
            ctxT_sb = cpool.tile([128, KT_C, S], F16)
            xT_sb = cpool.tile([128, KT_E, S], F16)

            qT_sb = qpool.tile([128, 2, S], F16)
            kT_sb = qpool.tile([128, 2, S], F16)
            # per (t, head): 128 cols = [ones (64) | v_h (64)] so one matmul
            # yields the replicated softmax denominator on PSUM rows 0:64
            # (partition base 0, as the custom reciprocal op requires) and
            # av rows 64:128 in a single PSUM bank
            v2_sb = qpool.tile([128, TT, HPG, 128], F16)
            avT_sb = qpool.tile([128, 2, S], F16)
            # ones halves of v2; on GpSimd so it gates nothing else
            nc.gpsimd.memset(v2_sb[:], 1.0)
            nc.vector.memset(warm_sb[:], 0.0)
            # pull the exp table load off the critical path
            nc.scalar.activation(warm_sb[:], warm_sb[:], AF.Exp)

            # ---- input DMAs: weights first (small), then x/ctx in column
            # groups so the first group's 14 tiles ride 14 HW queues ----
            for k in range(KT_C):
                nc.sync.dma_start(wk_sb[:, k, :], wk_r[:, k, :])
            nc.sync.dma_start(bk_sb[:], bk[:])
            for k in range(KT_C):
                nc.sync.dma_start(wv_sb[:, k, :], wv_r[:, k, :])
            nc.sync.dma_start(bv_sb[:], bv[:])
            for k in range(KT_E):
                nc.sync.dma_start(wq_sb[:, k, :], wq_r[:, k, :])
            nc.sync.dma_start(bq_sb[:], bq[:])
            for cg in range(4):
                csl = slice(cg * 512, (cg + 1) * 512)
                for k in range(KT_C):
                    nc.sync.dma_start(ctxT_sb[:, k, csl], ctxT_r[:, k, csl])
                for k in range(KT_E):
                    nc.sync.dma_start(xT_sb[:, k, csl], xT_r[:, k, csl])
                if cg == 1:
                    for wg in range(2):
                        wsl = slice(wg * 512, (wg + 1) * 512)
                        for l in range(2):
                            nc.sync.dma_start(wo_sb[:, l, wsl],
                                              wo_r[:, l, wsl])

            # HAM warm-up: dependency-free matmul chain keeps the PE busy
            # through the preamble + input-DMA stream so the first real
            # matmuls run at 2.4GHz instead of 1.2
            wrm_in = cpool.tile([128, 256], F16)
            nc.vector.memset(wrm_in[:], 0.5)

            with (
                tc.tile_pool(name="psc", bufs=2, space="PSUM") as psc,
                tc.tile_pool(name="pacc", bufs=2, space="PSUM") as pacc,
                tc.tile_pool(name="pbg", bufs=2, space="PSUM") as pbg,
            ):
                wrm_ps = pbg.tile([128, 256], F32, tag="bg", name="wrmps")
                for i in range(30):
                    nc.tensor.matmul(wrm_ps[:], wrm_in[:, 0:128],
                                     wrm_in[:], start=(i == 0),
                                     stop=(i == 29))
                # ---- background-work generators (1 yield = 1-2 matmuls) ----
                def proj_gen(dst, w_sb, b_sb, src, nk, l, sc, nm):
                    # produces dst[:, l, sc*512:(sc+1)*512]
                    pss = pbg.tile([128, 512], F32, tag="bg",
                                   name=f"pj{nm}_{l}_{sc}")
                    for k in range(nk):
                        nc.tensor.matmul(
                            pss[:],
                            w_sb[:, k, l * 128:(l + 1) * 128],
                            src[:, k, sc * 512:(sc + 1) * 512],
                            start=(k == 0), stop=(k == nk - 1),
                        )
                        yield
                    nc.vector.tensor_tensor(
                        dst[:, l, sc * 512:(sc + 1) * 512],
                        pss[:],
                        b_sb[:, l:l + 1].to_broadcast([128, 512]),
                        OP.add,
                    )
                    yield

                def v_gen(t, half):
                    # v2[:, t, 2h:2h+2, 0:64] = ctx_t @ wv[:, half] + bv
                    ps = pbg.tile([128, 128], F32, tag="bg",
                                  name=f"vps{t}_{half}")
                    for k in range(KT_C):
                        nc.tensor.matmul(
                            ps[:],
                            ctxT_sb[:, k, t * 128:(t + 1) * 128],
                            wv_sb[:, k, half * 128:(half + 1) * 128],
                            start=(k == 0), stop=(k == KT_C - 1),
                        )
                        if k % 2 == 1:
                            yield
                    nc.vector.tensor_tensor(
                        v2_sb[:, t, 2 * half:2 * half + 2, DH:128],
                        ps[:].rearrange("p (g d) -> p g d", d=DH),
                        bv_sb[:, half * 128:(half + 1) * 128].rearrange(
                            "p (g d) -> p g d", d=DH),
                        OP.add,
                    )
                    yield

                def outproj_gen(sc):
                    for st in range(4):
                        row = (sc * 4 + st) * 128
                        for n in range(2):
                            ps = pbg.tile([128, 512], F32, tag="bg",
                                          name=f"po{sc}_{st}_{n}")
                            for l in range(2):
                                nc.tensor.matmul(
                                    ps[:],
                                    avT_sb[:, l, row:row + 128],
                                    wo_sb[:, l, n * 512:(n + 1) * 512],
                                    start=(l == 0), stop=(l == 1),
                                )
                            yield
                            os_sb = ospool.tile([128, 512], F16, tag="os")
                            nc.vector.tensor_copy(os_sb[:], ps[:])
                            for half in range(2):
                                hsl = slice(n * 512 + half * 256,
                                            n * 512 + (half + 1) * 256)
                                nc.sync.dma_start(
                                    out[row:row + 128, hsl],
                                    os_sb[:, half * 256:(half + 1) * 256],
                                )
                            yield

                # work items: [deadline_T, generator, quanta_left]
                # deadline_T in linear iteration time T = ci*16 + t; the gen
                # must be fully emitted before iteration T begins.
                work = []
                for sc in range(SCK):
                    work.append([4 * sc,
                                 proj_gen(kT_sb, wk_sb, bk_sb, ctxT_sb,
                                          KT_C, 0, sc, "k"), KT_C + 1])
                for t in range(TT):
                    work.append([t, v_gen(t, 0), 4])
                for sc in range(SCK):
                    work.append([max(0, 16 * sc - 8),
                                 proj_gen(qT_sb, wq_sb, bq_sb, xT_sb,
                                          KT_E, 0, sc, "q"), KT_E + 1])
                # l1 work pulled into chunks 1-3 so chunk 4 isn't an
                # avalanche of forced drains
                for sc in range(SCK):
                    work.append([18 + 10 * sc,
                                 proj_gen(kT_sb, wk_sb, bk_sb, ctxT_sb,
                                          KT_C, 1, sc, "k"), KT_C + 1])
                for t in range(TT):
                    work.append([20 + (5 * t + 1) // 2, v_gen(t, 1), 4])
                for sc in range(SCK):
                    work.append([52 + 16 * sc,
                                 proj_gen(qT_sb, wq_sb, bq_sb, xT_sb,
                                          KT_E, 1, sc, "q"), KT_E + 1])

                def advance(w, n):
                    for _ in range(n):
                        try:
                            next(w[1])
                            w[2] -= 1
                        except StopIteration:
                            w[2] = 0
                            break
                    if w[2] <= 0:
                        try:
                            for _ in w[1]:
                                pass
                        except StopIteration:
                            pass
                        return False
                    return True

                def drive(T):
                    # just-in-time: keep every gen's remaining quanta within
                    # its slack (1 quantum/iter burn rate), earliest first
                    work.sort(key=lambda w: w[0])
                    for w in list(work):
                        slack = w[0] - T - 1
                        if slack <= 0:
                            advance(w, 10 ** 6)
                            work.remove(w)
                        elif w[2] > slack:
                            if not advance(w, w[2] - slack):
                                work.remove(w)

                def drive_one(T):
                    if work:
                        if not advance(work[0], 1):
                            work.remove(work[0])

                # ---- main p-major chunk loop -------------------------------
                for ci, (p, sc) in enumerate(
                        [(p, sc) for p in (0, 1) for sc in range(SCK)]):
                    ssl = slice(sc * 512, (sc + 1) * 512)
                    avz = {h: pacc.tile([128, 512], F32, tag="acc",
                                        name=f"avz{ci}_{h}")
                           for h in range(2)}
                    exq = []

                    def emit_av(tt_):
                        ex_ = exq.pop(0)
                        for h in range(2):
                            nc.tensor.matmul(
                                avz[h][:, :],
                                v2_sb[:, tt_, p * 2 + h, :],
                                ex_[:, h * 512:(h + 1) * 512],
                                start=(tt_ == 0), stop=(tt_ == TT - 1),
                            )

                    # 2-iteration blocks: [sc,sc][av,av,bg...] groups matmuls
                    # by PE row mode — half-row scores pairs chain their
                    # LDWEIGHTS under each other (disjoint row groups) and
                    # full-row av/bg matmuls chain via the shadow weight
                    # buffer; mode transitions (which expose a serial LDW)
                    # drop from 4 to 2 per two iterations.
                    for tb in range(0, TT, 2):
                        drive(ci * 16 + tb)
                        scps = []
                        for t in (tb, tb + 1):
                            scp = psc.tile([128, 1024], F32, tag="sc",
                                           name=f"sc{ci}_{t}")
                            for h in range(2):
                                hb = h * DH
                                nc.tensor.matmul(
                                    scp[:, h * 512:(h + 1) * 512],
                                    kT_sb[hb:hb + DH, p,
                                          t * 128:(t + 1) * 128],
                                    qT_sb[hb:hb + DH, p, ssl],
                                    start=True, stop=True,
                                )
                            scps.append(scp)
                        for i, t in enumerate((tb, tb + 1)):
                            ex = expool.tile([128, 1024], F16, tag="ex",
                                             name=f"ex{ci}_{t}")
                            nc.scalar.activation(ex[:], scps[i][:], AF.Exp,
                                                 scale=0.125)
                            exq.append(ex)
                        if tb >= 2:
                            emit_av(tb - 2)
                            emit_av(tb - 1)
                        drive_one(ci * 16 + tb)
                        drive_one(ci * 16 + tb + 1)
                    emit_av(TT - 2)
                    emit_av(TT - 1)
                    for h in range(2):
                        hb = h * DH
                        # custom DVE op: input at partition base 0 (Z rows)
                        rz = ospool.tile([64, 512], F32, tag="rz",
                                         name=f"rz{ci}_{h}")
                        nc.vector.reciprocal_approx_fast(
                            rz[:], avz[h][0:DH, :])
                        nc.vector.tensor_tensor(
                            avT_sb[hb:hb + DH, p, ssl],
                            avz[h][DH:128, :],
                            rz[:],
                            OP.mult,
                        )
                    if p == 1:
                        # avT rows for this sc now complete in both l slices
                        dl = (5 + sc) * 16 + 15 if sc < 3 else 10 ** 6
                        work.append([dl, outproj_gen(sc), 16])
                # drain remaining outproj work (tail)
                for w in list(work):
                    advance(w, 10 ** 6)

    nc.compile()
    return nc


def get_nc():
    if "nc" not in _NC_CACHE:
        _NC_CACHE["nc"] = _build_nc()
    return _NC_CACHE["nc"]


def make_in_maps(x, context, Wq, bq, Wk, bk, Wv, bv, Wo, bo):
    x = np.asarray(x, dtype=np.float32)
    context = np.asarray(context, dtype=np.float32)
    Wq = np.asarray(Wq, dtype=np.float32)
    Wk = np.asarray(Wk, dtype=np.float32)
    Wv = np.asarray(Wv, dtype=np.float32)
    Wo = np.asarray(Wo, dtype=np.float32)
    bq = np.asarray(bq, dtype=np.float32)
    bk = np.asarray(bk, dtype=np.float32)
    bv = np.asarray(bv, dtype=np.float32)

    xT = [np.ascontiguousarray(x[b].T).astype(np.float16) for b in range(B)]
    ctxT = [np.ascontiguousarray(context[b].T).astype(np.float16)
            for b in range(B)]
    in_maps = []
    for c in range(N_CORES):
        b, g = c // GROUPS, c % GROUPS
        sl = slice(g * DSL, (g + 1) * DSL)
        in_maps.append({
            "xT": xT[b],
            "ctxT": ctxT[b],
            "wq": Wq[:, sl].astype(np.float16),
            "wk": Wk[:, sl].astype(np.float16),
            "wv": Wv[:, sl].astype(np.float16),
            "wo": Wo[sl, :].astype(np.float16),
            "bq": np.ascontiguousarray(bq[sl].reshape(2, 128).T),
            "bk": np.ascontiguousarray(bk[sl].reshape(2, 128).T),
            "bv": np.tile(bv[sl].reshape(1, DSL).astype(np.float32),
                          (128, 1)),
        })
    return in_maps


def run_sharded(inputs, trace=False):
    nc = get_nc()
    in_maps = make_in_maps(**inputs)
    res = bass_utils.run_bass_kernel_spmd(
        nc, in_maps, core_ids=list(range(N_CORES)), trace=trace,
    )
    bo = np.asarray(inputs["bo"], dtype=np.float32)
    full = np.empty((B, S, E), dtype=np.float32)
    for b in range(B):
        acc = res.results[b * GROUPS]["out"].astype(np.float32)
        for g in range(1, GROUPS):
            acc = acc + res.results[b * GROUPS + g]["out"].astype(np.float32)
        full[b] = acc + bo[None, :]
    return full, res.exec_time_ns


def kernel(**inputs) -> np.ndarray:
    return run_sharded(inputs)[0]
